# revision 5
# baseline (speedup 1.0000x reference)
"""Rowwise cosine-similarity kernel for Trainium2 (8 NeuronCores, SPMD).

Computes out[b, n] = sum_d(an * bn) where an, bn are L2-normalized rows of
a, b [16, 4096, 256] -> out [16, 4096].

Sharding: 65536 rows split across 8 cores (8192 rows/core). Per core the
row slice is viewed as [128 partitions, 64 subtiles, 256], i.e. row
p*64 + t lives at partition p, subtile t. Everything is contiguous DMA.

Per 256-wide subtile the kernel needs three reductions over d:
  P  = sum(a*b),  Sa = sum(a*a),  Sb = sum(b*b)
then out = P * sqrt(1/(Sa*Sb)).

Engine split (the DMA stream, 16.8 MB/core at ~420 GB/s ~= 40.5 us, is the
roofline; every engine must stay under it):
  DVE : P via fused scalar_tensor_tensor (f32, 1 elem/cyc), a few Sa
        squares, plus segmented bf16 tensor_reduce of the Pool squares
        (2 elem/cyc in the 2x_1P perf mode - bf16 in AND out required).
  ACT : most Sa squares via Square+accumulate; finalize Sqrt + casts.
  Pool: (GpSimd) chunk-wide tensor_tensor b*b squares with bf16 output.
        Pool cannot reduce f32, so DVE picks up the cheap 2x reduce.
Sb only needs ~1e-3 relative accuracy (it feeds the normalization, so its
relative error passes straight through); bf16 is plenty. P must stay f32.

The last chunks are tapered and skip the Pool hop so the tail after the
final DMA bytes is short.
"""

import sys

if "/opt/trn_rl_repo" not in sys.path:
    sys.path.insert(0, "/opt/trn_rl_repo")

import numpy as np
import orjson

import concourse.bass as bass
import concourse.mybir as mybir
import concourse.tile as tile
from concourse import bass2jax, bass_utils

# ---------------------------------------------------------------------------
# Environment patches.
#
# 1. No cloud share in this sandbox: upload_artifacts would fail.
# 2. The walrus build here accepts at most ONE semaphore wait per
#    instruction; the Tile scheduler freely attaches several.  Post-process
#    the BIR before compiling: move surplus waits onto single-wait Drain
#    carrier instructions inserted just before the original instruction on
#    the same engine queue.
# ---------------------------------------------------------------------------

bass_utils.upload_artifacts = lambda tmpdir: ""

_MAX_WAITS = 1
REMOVE_PE = True


def _split_bir_waits(bir_json: bytes) -> bytes:
    d = orjson.loads(bir_json)
    ctr = 0
    for fn in d.get("functions", []):
        for blk in fn.get("blocks", []):
            insts = blk.get("instructions")
            if not insts:
                continue
            out = []
            for inst in insts:
                si = inst.get("sync_info")
                waits = (si or {}).get("on_wait") or []
                if len(waits) > _MAX_WAITS:
                    surplus = waits[:-_MAX_WAITS]
                    si["on_wait"] = waits[-_MAX_WAITS:]
                    for i in range(0, len(surplus), _MAX_WAITS):
                        out.append(
                            {
                                "name": f"WSPL-{ctr}",
                                "opcode": "Drain",
                                "engine": inst["engine"],
                                "ins": [],
                                "outs": [],
                                "is_reset_sema": False,
                                "debug": inst.get("debug", 0),
                                "sync_info": {
                                    "on_wait": surplus[i : i + _MAX_WAITS],
                                    "on_update": [],
                                },
                            }
                        )
                        ctr += 1
                out.append(inst)
            blk["instructions"] = out
    return orjson.dumps(d)


def _exempt_sp_from_entry_barrier(d: dict) -> None:
    """Let the SP (DMA-issuing) engine skip the kernel-entry barrier.

    The entry barrier only protects the const-AP memsets, which SP never
    reads; removing SP's blocking wait lets input DMAs start ~5 us earlier.
    The leader's release count is reduced so both sems still return to 0.
    """
    blk = d["functions"][0]["blocks"][0]
    insts = blk["instructions"]
    if not any(i.get("opcode") == "Memset" for i in insts):
        return
    sp_idx = None
    pool_add = None
    for i, inst in enumerate(insts):
        if inst.get("opcode") != "EventSemaphore":
            continue
        si = inst.get("sync_info") or {}
        ow = si.get("on_wait") or []
        ou = si.get("on_update") or []
        if not ou:
            continue
        u0 = ou[0]
        if "release" not in str(u0.get("ant_name", "")):
            continue
        if inst.get("engine") == "SP" and u0.get("update_mode") == "sem-dec":
            sp_idx = i
        if (
            inst.get("engine") == "Pool"
            and not ow
            and u0.get("update_mode") == "sem-add-imm"
        ):
            pool_add = inst
    if sp_idx is not None and pool_add is not None:
        uv = pool_add["sync_info"]["on_update"][0]
        if uv["update_value"] >= 2:
            del insts[sp_idx]
            uv["update_value"] -= 1


def _remove_pe_instructions(d: dict) -> None:
    """Drop every PE instruction from the BIR.

    This kernel never uses the tensor engine, but bass still emits barrier
    participation for it; the NEFF prolog then waits ~2.5 us for PE's
    HW-decode instruction stream to DMA in before the first barrier
    releases.  Removing PE from the program (and fixing the two barrier
    sems' counts) sidesteps that.
    """
    for fn in d.get("functions", []):
        for blk in fn.get("blocks", []):
            insts = blk.get("instructions") or []
            if not any(i.get("engine") == "PE" for i in insts):
                continue
            pe_gather = 0
            pe_release_waiters = 0
            for i in insts:
                if i.get("engine") != "PE":
                    continue
                si = i.get("sync_info") or {}
                for u in si.get("on_update") or []:
                    if "gather" in str(u.get("ant_name", "")):
                        pe_gather += 1
                if i.get("opcode") == "EventSemaphore":
                    for w in si.get("on_wait") or []:
                        if "release" in str(w.get("ant_name", "")):
                            pe_release_waiters += 1
            new = [i for i in insts if i.get("engine") != "PE"]
            for i in new:
                si = i.get("sync_info") or {}
                for w in si.get("on_wait") or []:
                    if (
                        "gather" in str(w.get("ant_name", ""))
                        and w.get("wait_mode") == "sem-ge-imm"
                    ):
                        w["wait_value"] -= pe_gather
                for u in si.get("on_update") or []:
                    if (
                        "gather" in str(u.get("ant_name", ""))
                        and u.get("update_mode") == "sem-sub-imm"
                    ):
                        u["update_value"] -= pe_gather
                    if (
                        "release" in str(u.get("ant_name", ""))
                        and u.get("update_mode") == "sem-add-imm"
                    ):
                        u["update_value"] -= pe_release_waiters
            blk["instructions"] = new


_orig_compile_bir_kernel = bass_utils.compile_bir_kernel


def _patched_compile_bir_kernel(bir_json, tmpdir, neff_name="file.neff"):
    if isinstance(bir_json, str):
        bir_json = bir_json.encode()
    d = orjson.loads(_split_bir_waits(bir_json))
    _exempt_sp_from_entry_barrier(d)
    if REMOVE_PE:
        _remove_pe_instructions(d)
    bir_json = orjson.dumps(d)
    return _orig_compile_bir_kernel(bir_json, tmpdir, neff_name=neff_name)


bass_utils.compile_bir_kernel = _patched_compile_bir_kernel
bass2jax.compile_bir_kernel = _patched_compile_bir_kernel

from concourse.vector_clock import ScopedClock  # noqa: E402


def _lean_drain_and_barrier(self, tick_clock, wait_clock):
    """Tile kernel tail without the trailing all-engine barrier.

    After the first barrier every engine is done with real work; gpsimd
    clears the semaphores and each engine halts independently (NRT waits
    for all engines anyway), so the second barrier only adds latency.
    """
    drain_inst = self.nc.sync.drain()
    wait_clock.add_sem_waits(
        drain_inst.ins, ScopedClock({None: tick_clock.global_clock})
    )
    self.nc.all_engine_barrier()
    popped = self.nc._tile_sem_poison_stack.pop()
    assert popped is self._sem_poison
    self.nc.clear_and_free_semaphores(list(self.sems.allocated().values()))


tile.TileContext._drain_and_barrier = _lean_drain_and_barrier

# ---------------------------------------------------------------------------
# Problem constants (hardcoded; kernel.py must be self-contained).
# ---------------------------------------------------------------------------

N_CORES = 8
B, N, D = 16, 4096, 256
ROWS = B * N                     # 65536
ROWS_PER_CORE = ROWS // N_CORES  # 8192
P = 128                          # SBUF partitions
T = ROWS_PER_CORE // P           # 64 subtiles per core
COLS = T * D                     # 16384 dram cols per partition
# Chunk sizes in subtiles. Uniform 8s while streaming; tapered at the end
# so the work that depends on the final bytes is tiny.
CHUNK_PLAN = (8, 8, 8, 8, 8, 8, 8, 4, 2, 2)
assert sum(CHUNK_PLAN) == T
# Chunks whose Sb squares run on Pool (GpSimd); tail chunks skip the Pool
# hop so the post-stream dependency chain is short.
POOL_CHUNKS = 7
# Finalize boundaries (tbase values): emit out[lo:tb] when tbase hits these.
FIN_BOUNDS = (56, 62, 64)
CHUNK_BUFS = 8
BSQ_BUFS = 3

# Sa squares (64) + tail Sb squares (8) are split DVE/ACT.  Measured
# per-op: DVE stt ~344 ns, ACT square+accum ~596 ns.
N_DVE_SQ = 16


def _sq_on_dve(idx: int, total: int) -> bool:
    return (idx * N_DVE_SQ) // total != ((idx + 1) * N_DVE_SQ) // total


_CACHE: dict = {}


def _build_bass():
    f32 = mybir.dt.float32
    bf16 = mybir.dt.bfloat16
    alu = mybir.AluOpType
    act = mybir.ActivationFunctionType

    nc = bass.Bass(
        "TRN2",
        debug=False,
        num_devices=N_CORES,
        enable_asserts=False,
        enable_partition_id=False,
    )
    a_d = nc.dram_tensor("a", (P, COLS), f32, kind="ExternalInput").ap()
    b_d = nc.dram_tensor("b", (P, COLS), f32, kind="ExternalInput").ap()
    o_d = nc.dram_tensor("out", (P, T), f32, kind="ExternalOutput").ap()

    # Count non-Pool squares for the DVE/ACT interleave.
    n_tail_sq = sum(
        ct for i, ct in enumerate(CHUNK_PLAN) if i >= POOL_CHUNKS
    )
    total_split_sq = T + n_tail_sq

    with tile.TileContext(nc) as tc, nc.allow_low_precision(
        reason="Sb feeds the normalization; bf16 (~1e-3 rel) is ample"
    ):
        with (
            tc.tile_pool(name="stats", bufs=1) as stats_pool,
            tc.tile_pool(name="chunks", bufs=CHUNK_BUFS) as chunk_pool,
            tc.tile_pool(name="bsq", bufs=BSQ_BUFS) as bsq_pool,
            tc.tile_pool(name="dscr", bufs=2) as dve_scr,
            tc.tile_pool(name="ascr", bufs=2) as act_scr,
            tc.tile_pool(name="fin", bufs=1) as fin_pool,
        ):
            p_t = stats_pool.tile([P, T], f32, tag="p")
            sa_t = stats_pool.tile([P, T], f32, tag="sa")
            sb_t = stats_pool.tile([P, T], bf16, tag="sb")
            sbf = fin_pool.tile([P, T], f32, tag="sbf")
            denom = fin_pool.tile([P, T], f32, tag="denom")
            rec = fin_pool.tile([P, T], f32, tag="rec")
            rsq = fin_pool.tile([P, T], f32, tag="rsq")
            out_t = fin_pool.tile([P, T], f32, tag="out")

            sq_idx = 0  # running index over DVE/ACT-split squares
            fin_lo = 0
            tbase = 0
            for ci, chunk_t in enumerate(CHUNK_PLAN):
                c0 = tbase * D
                c1 = (tbase + chunk_t) * D
                a_ch = chunk_pool.tile([P, CHUNK_PLAN[0] * D], f32, tag="a")
                b_ch = chunk_pool.tile([P, CHUNK_PLAN[0] * D], f32, tag="b")
                nc.sync.dma_start(a_ch[:, : chunk_t * D], a_d[:, c0:c1])
                nc.sync.dma_start(b_ch[:, : chunk_t * D], b_d[:, c0:c1])

                use_pool = ci < POOL_CHUNKS
                if use_pool:
                    # Sb for the whole chunk: Pool squares (bf16 out), DVE
                    # segmented-reduces at 2 elem/cyc (bf16 in+out).
                    bsq = bsq_pool.tile([P, CHUNK_PLAN[0] * D], bf16, tag="bsq")
                    nc.gpsimd.tensor_tensor(
                        out=bsq[:, : chunk_t * D],
                        in0=b_ch[:, : chunk_t * D],
                        in1=b_ch[:, : chunk_t * D],
                        op=alu.mult,
                    )
                    nc.vector.tensor_reduce(
                        out=sb_t[:, tbase : tbase + chunk_t],
                        in_=bsq[:, : chunk_t * D].rearrange(
                            "p (t d) -> p t d", d=D
                        ),
                        axis=mybir.AxisListType.X,
                        op=alu.add,
                    )

                for s in range(chunk_t):
                    t = tbase + s
                    asub = a_ch[:, s * D : (s + 1) * D]
                    bsub = b_ch[:, s * D : (s + 1) * D]

                    # P: fused multiply + accum-reduce on DVE (f32).
                    prod = dve_scr.tile([P, D], f32, tag="prod")
                    nc.vector.scalar_tensor_tensor(
                        out=prod[:],
                        in0=asub,
                        scalar=0.0,
                        in1=bsub,
                        op0=alu.add,
                        op1=alu.mult,
                        accum_out=p_t[:, t : t + 1],
                    )

                    # Sa always; Sb only for tail (non-Pool) chunks.
                    pairs = [(asub, sa_t)]
                    if not use_pool:
                        pairs.append((bsub, sb_t))
                    for sub, dst in pairs:
                        if _sq_on_dve(sq_idx, total_split_sq):
                            scr = dve_scr.tile([P, D], f32, tag="dsq")
                            nc.vector.scalar_tensor_tensor(
                                out=scr[:],
                                in0=sub,
                                scalar=0.0,
                                in1=sub,
                                op0=alu.add,
                                op1=alu.mult,
                                accum_out=dst[:, t : t + 1],
                            )
                        else:
                            scr = act_scr.tile([P, D], f32, tag="asq")
                            nc.scalar.activation(
                                scr[:], sub, act.Square, accum_out=dst[:, t : t + 1]
                            )
                        sq_idx += 1

                tbase += chunk_t

                # Finalize ready column ranges early so only the last
                # chunk's finalize sits in the tail:
                #   out = P * sqrt(1 / (Sa * Sb)).
                if tbase in FIN_BOUNDS:
                    lo, hi = fin_lo, tbase
                    nc.scalar.copy(sbf[:, lo:hi], sb_t[:, lo:hi])
                    nc.vector.tensor_mul(
                        denom[:, lo:hi], sa_t[:, lo:hi], sbf[:, lo:hi]
                    )
                    nc.vector.reciprocal(rec[:, lo:hi], denom[:, lo:hi])
                    nc.scalar.activation(rsq[:, lo:hi], rec[:, lo:hi], act.Sqrt)
                    nc.vector.tensor_mul(
                        out_t[:, lo:hi], p_t[:, lo:hi], rsq[:, lo:hi]
                    )
                    nc.sync.dma_start(o_d[:, lo:hi], out_t[:, lo:hi])
                    fin_lo = tbase

    return nc


def _get_nc():
    if "nc" not in _CACHE:
        _CACHE["nc"] = _build_bass()
    return _CACHE["nc"]


def kernel(a: np.ndarray, b: np.ndarray) -> np.ndarray:
    a = np.ascontiguousarray(np.asarray(a, dtype=np.float32)).reshape(ROWS, D)
    b = np.ascontiguousarray(np.asarray(b, dtype=np.float32)).reshape(ROWS, D)

    in_maps = []
    for c in range(N_CORES):
        sl = slice(c * ROWS_PER_CORE, (c + 1) * ROWS_PER_CORE)
        in_maps.append(
            {"a": a[sl].reshape(P, COLS), "b": b[sl].reshape(P, COLS)}
        )

    nc = _get_nc()
    res = bass_utils.run_bass_kernel_spmd(nc, in_maps, core_ids=list(range(N_CORES)))
    out = np.concatenate(
        [res.results[c]["out"].reshape(ROWS_PER_CORE) for c in range(N_CORES)]
    )
    return out.reshape(B, N)


# revision 9
# speedup vs baseline: 1.2919x; 1.2919x over previous
"""Rowwise cosine-similarity kernel for Trainium2 (8 NeuronCores, SPMD).

Computes out[b, n] = sum_d(an * bn) where an, bn are L2-normalized rows of
a, b [16, 4096, 256] -> out [16, 4096].

Sharding: 65536 rows split across 8 cores (8192 rows/core). Per core the
row slice is viewed as [128 partitions, 64 subtiles, 256], i.e. row
p*64 + t lives at partition p, subtile t. Everything is contiguous DMA.

Per 256-wide subtile the kernel needs three reductions over d:
  P  = sum(a*b),  Sa = sum(a*a),  Sb = sum(b*b)
then out = P * sqrt(1/(Sa*Sb)).

Engine split (the DMA stream, 16.8 MB/core at ~420 GB/s ~= 40.5 us, is the
roofline; every engine must stay under it):
  DVE : P via fused scalar_tensor_tensor (f32, 1 elem/cyc), a few Sa
        squares, plus segmented bf16 tensor_reduce of the Pool squares
        (2 elem/cyc in the 2x_1P perf mode - bf16 in AND out required).
  ACT : most Sa squares via Square+accumulate; finalize Sqrt + casts.
  Pool: (GpSimd) chunk-wide tensor_tensor b*b squares with bf16 output.
        Pool cannot reduce f32, so DVE picks up the cheap 2x reduce.
Sb only needs ~1e-3 relative accuracy (it feeds the normalization, so its
relative error passes straight through); bf16 is plenty. P must stay f32.

The last chunks are tapered and skip the Pool hop so the tail after the
final DMA bytes is short.
"""

import sys

if "/opt/trn_rl_repo" not in sys.path:
    sys.path.insert(0, "/opt/trn_rl_repo")

import numpy as np
import orjson

import concourse.bass as bass
import concourse.mybir as mybir
import concourse.tile as tile
from concourse import bass2jax, bass_utils

# ---------------------------------------------------------------------------
# Environment patches.
#
# 1. No cloud share in this sandbox: upload_artifacts would fail.
# 2. The walrus build here accepts at most ONE semaphore wait per
#    instruction; the Tile scheduler freely attaches several.  Post-process
#    the BIR before compiling: move surplus waits onto single-wait Drain
#    carrier instructions inserted just before the original instruction on
#    the same engine queue.
# ---------------------------------------------------------------------------

bass_utils.upload_artifacts = lambda tmpdir: ""

_MAX_WAITS = 1
REMOVE_PE = True


def _split_bir_waits(bir_json: bytes) -> bytes:
    d = orjson.loads(bir_json)
    ctr = 0
    for fn in d.get("functions", []):
        for blk in fn.get("blocks", []):
            insts = blk.get("instructions")
            if not insts:
                continue
            out = []
            for inst in insts:
                si = inst.get("sync_info")
                waits = (si or {}).get("on_wait") or []
                if len(waits) > _MAX_WAITS:
                    surplus = waits[:-_MAX_WAITS]
                    si["on_wait"] = waits[-_MAX_WAITS:]
                    for i in range(0, len(surplus), _MAX_WAITS):
                        out.append(
                            {
                                "name": f"WSPL-{ctr}",
                                "opcode": "Drain",
                                "engine": inst["engine"],
                                "ins": [],
                                "outs": [],
                                "is_reset_sema": False,
                                "debug": inst.get("debug", 0),
                                "sync_info": {
                                    "on_wait": surplus[i : i + _MAX_WAITS],
                                    "on_update": [],
                                },
                            }
                        )
                        ctr += 1
                out.append(inst)
            blk["instructions"] = out
    return orjson.dumps(d)


def _exempt_sp_from_entry_barrier(d: dict) -> None:
    """Let the SP (DMA-issuing) engine skip the kernel-entry barrier.

    The entry barrier only protects the const-AP memsets, which SP never
    reads; removing SP's blocking wait lets input DMAs start ~5 us earlier.
    The leader's release count is reduced so both sems still return to 0.
    """
    blk = d["functions"][0]["blocks"][0]
    insts = blk["instructions"]
    if not any(i.get("opcode") == "Memset" for i in insts):
        return
    sp_idx = None
    pool_add = None
    for i, inst in enumerate(insts):
        if inst.get("opcode") != "EventSemaphore":
            continue
        si = inst.get("sync_info") or {}
        ow = si.get("on_wait") or []
        ou = si.get("on_update") or []
        if not ou:
            continue
        u0 = ou[0]
        if "release" not in str(u0.get("ant_name", "")):
            continue
        if inst.get("engine") == "SP" and u0.get("update_mode") == "sem-dec":
            sp_idx = i
        if (
            inst.get("engine") == "Pool"
            and not ow
            and u0.get("update_mode") == "sem-add-imm"
        ):
            pool_add = inst
    if sp_idx is not None and pool_add is not None:
        uv = pool_add["sync_info"]["on_update"][0]
        if uv["update_value"] >= 2:
            del insts[sp_idx]
            uv["update_value"] -= 1


def _remove_pe_instructions(d: dict) -> None:
    """Drop every PE instruction from the BIR.

    This kernel never uses the tensor engine, but bass still emits barrier
    participation for it; the NEFF prolog then waits ~2.5 us for PE's
    HW-decode instruction stream to DMA in before the first barrier
    releases.  Removing PE from the program (and fixing the two barrier
    sems' counts) sidesteps that.
    """
    for fn in d.get("functions", []):
        for blk in fn.get("blocks", []):
            insts = blk.get("instructions") or []
            if not any(i.get("engine") == "PE" for i in insts):
                continue
            pe_gather = 0
            pe_release_waiters = 0
            for i in insts:
                if i.get("engine") != "PE":
                    continue
                si = i.get("sync_info") or {}
                for u in si.get("on_update") or []:
                    if "gather" in str(u.get("ant_name", "")):
                        pe_gather += 1
                if i.get("opcode") == "EventSemaphore":
                    for w in si.get("on_wait") or []:
                        if "release" in str(w.get("ant_name", "")):
                            pe_release_waiters += 1
            new = [i for i in insts if i.get("engine") != "PE"]
            for i in new:
                si = i.get("sync_info") or {}
                for w in si.get("on_wait") or []:
                    if (
                        "gather" in str(w.get("ant_name", ""))
                        and w.get("wait_mode") == "sem-ge-imm"
                    ):
                        w["wait_value"] -= pe_gather
                for u in si.get("on_update") or []:
                    if (
                        "gather" in str(u.get("ant_name", ""))
                        and u.get("update_mode") == "sem-sub-imm"
                    ):
                        u["update_value"] -= pe_gather
                    if (
                        "release" in str(u.get("ant_name", ""))
                        and u.get("update_mode") == "sem-add-imm"
                    ):
                        u["update_value"] -= pe_release_waiters
            blk["instructions"] = new


_orig_compile_bir_kernel = bass_utils.compile_bir_kernel


def _patched_compile_bir_kernel(bir_json, tmpdir, neff_name="file.neff"):
    if isinstance(bir_json, str):
        bir_json = bir_json.encode()
    d = orjson.loads(_split_bir_waits(bir_json))
    _exempt_sp_from_entry_barrier(d)
    if REMOVE_PE:
        _remove_pe_instructions(d)
    bir_json = orjson.dumps(d)
    return _orig_compile_bir_kernel(bir_json, tmpdir, neff_name=neff_name)


bass_utils.compile_bir_kernel = _patched_compile_bir_kernel
bass2jax.compile_bir_kernel = _patched_compile_bir_kernel

from concourse.vector_clock import ScopedClock  # noqa: E402


def _lean_drain_and_barrier(self, tick_clock, wait_clock):
    """Tile kernel tail without the trailing all-engine barrier.

    After the first barrier every engine is done with real work; gpsimd
    clears the semaphores and each engine halts independently (NRT waits
    for all engines anyway), so the second barrier only adds latency.
    """
    drain_inst = self.nc.sync.drain()
    wait_clock.add_sem_waits(
        drain_inst.ins, ScopedClock({None: tick_clock.global_clock})
    )
    self.nc.all_engine_barrier()
    popped = self.nc._tile_sem_poison_stack.pop()
    assert popped is self._sem_poison
    self.nc.clear_and_free_semaphores(list(self.sems.allocated().values()))


tile.TileContext._drain_and_barrier = _lean_drain_and_barrier

# ---------------------------------------------------------------------------
# Problem constants (hardcoded; kernel.py must be self-contained).
# ---------------------------------------------------------------------------

N_CORES = 8
B, N, D = 16, 4096, 256
ROWS = B * N                     # 65536
ROWS_PER_CORE = ROWS // N_CORES  # 8192
P = 128                          # SBUF partitions
T = ROWS_PER_CORE // P           # 64 subtiles per core
COLS = T * D                     # 16384 dram cols per partition
# Chunk sizes in subtiles. Uniform 8s while streaming; tapered at the end
# so the work that depends on the final bytes is tiny.
CHUNK_PLAN = (8, 8, 8, 8, 8, 8, 8, 4, 2, 2)
assert sum(CHUNK_PLAN) == T
# Chunks whose Sb squares run on Pool (GpSimd).  DISABLED (0): Pool's TT
# holds the shared SBUF port pair that DVE's 2-source stt also needs, so
# they serialize instead of overlapping (measured: stt slices stretch to
# 3.7 us while a Pool chunk-TT holds the port).
POOL_CHUNKS = 0
# Finalize boundaries (tbase values): emit out[lo:tb] when tbase hits these.
FIN_BOUNDS = (56, 62, 64)
CHUNK_BUFS = 8
BSQ_BUFS = 3
# First FILL_T subtiles of chunk 0 get their own small DMAs so compute
# starts as soon as they land instead of waiting for the full 2 MB pair.
FILL_T = 2

# The 128 square ops are split DVE/ACT.  Measured per-op: DVE stt ~344 ns,
# ACT square+accum ~596 ns (~554 with PSUM scratch).  Balance point:
# 22016 + 344x = 554(128-x) -> x ~= 54.
N_DVE_SQ = 54
ACT_SQ_PSUM = True


def _sq_on_dve(idx: int, total: int) -> bool:
    return (idx * N_DVE_SQ) // total != ((idx + 1) * N_DVE_SQ) // total


_CACHE: dict = {}


def _build_bass():
    f32 = mybir.dt.float32
    bf16 = mybir.dt.bfloat16
    alu = mybir.AluOpType
    act = mybir.ActivationFunctionType

    nc = bass.Bass(
        "TRN2",
        debug=False,
        num_devices=N_CORES,
        enable_asserts=False,
        enable_partition_id=False,
    )
    a_d = nc.dram_tensor("a", (P, COLS), f32, kind="ExternalInput").ap()
    b_d = nc.dram_tensor("b", (P, COLS), f32, kind="ExternalInput").ap()
    o_d = nc.dram_tensor("out", (P, T), f32, kind="ExternalOutput").ap()

    # Count non-Pool squares for the DVE/ACT interleave.
    n_pool_sq = sum(
        ct for i, ct in enumerate(CHUNK_PLAN) if i < POOL_CHUNKS
    )
    total_split_sq = 2 * T - n_pool_sq

    with tile.TileContext(nc) as tc, nc.allow_low_precision(
        reason="Sb feeds the normalization; bf16 (~1e-3 rel) is ample"
    ):
        with (
            tc.tile_pool(name="stats", bufs=1) as stats_pool,
            tc.tile_pool(name="chunks", bufs=CHUNK_BUFS) as chunk_pool,
            tc.tile_pool(name="bsq", bufs=BSQ_BUFS) as bsq_pool,
            tc.tile_pool(name="dscr", bufs=2) as dve_scr,
            tc.tile_pool(name="ascr", bufs=2) as act_scr,
            tc.tile_pool(name="apsc", bufs=2, space="PSUM") as act_psum,
            tc.tile_pool(name="fin", bufs=1) as fin_pool,
        ):
            p_t = stats_pool.tile([P, T], f32, tag="p")
            sa_t = stats_pool.tile([P, T], f32, tag="sa")
            sb_dt = bf16 if POOL_CHUNKS else f32
            sb_t = stats_pool.tile([P, T], sb_dt, tag="sb")
            sbf = fin_pool.tile([P, T], f32, tag="sbf")
            denom = fin_pool.tile([P, T], f32, tag="denom")
            rec = fin_pool.tile([P, T], f32, tag="rec")
            rsq = fin_pool.tile([P, T], f32, tag="rsq")
            out_t = fin_pool.tile([P, T], f32, tag="out")

            sq_idx = 0  # running index over DVE/ACT-split squares
            fin_lo = 0
            tbase = 0
            for ci, chunk_t in enumerate(CHUNK_PLAN):
                c0 = tbase * D
                c1 = (tbase + chunk_t) * D
                a_ch = chunk_pool.tile([P, CHUNK_PLAN[0] * D], f32, tag="a")
                b_ch = chunk_pool.tile([P, CHUNK_PLAN[0] * D], f32, tag="b")
                if ci == 0 and FILL_T:
                    fs = FILL_T * D
                    nc.sync.dma_start(a_ch[:, :fs], a_d[:, c0 : c0 + fs])
                    nc.sync.dma_start(b_ch[:, :fs], b_d[:, c0 : c0 + fs])
                    nc.sync.dma_start(a_ch[:, fs : chunk_t * D], a_d[:, c0 + fs : c1])
                    nc.sync.dma_start(b_ch[:, fs : chunk_t * D], b_d[:, c0 + fs : c1])
                else:
                    nc.sync.dma_start(a_ch[:, : chunk_t * D], a_d[:, c0:c1])
                    nc.sync.dma_start(b_ch[:, : chunk_t * D], b_d[:, c0:c1])

                use_pool = ci < POOL_CHUNKS
                if use_pool:
                    # Sb for the whole chunk: Pool squares (bf16 out), DVE
                    # segmented-reduces at 2 elem/cyc (bf16 in+out).
                    bsq = bsq_pool.tile([P, CHUNK_PLAN[0] * D], bf16, tag="bsq")
                    nc.gpsimd.tensor_tensor(
                        out=bsq[:, : chunk_t * D],
                        in0=b_ch[:, : chunk_t * D],
                        in1=b_ch[:, : chunk_t * D],
                        op=alu.mult,
                    )
                    nc.vector.tensor_reduce(
                        out=sb_t[:, tbase : tbase + chunk_t],
                        in_=bsq[:, : chunk_t * D].rearrange(
                            "p (t d) -> p t d", d=D
                        ),
                        axis=mybir.AxisListType.X,
                        op=alu.add,
                    )

                for s in range(chunk_t):
                    t = tbase + s
                    asub = a_ch[:, s * D : (s + 1) * D]
                    bsub = b_ch[:, s * D : (s + 1) * D]

                    # P: fused multiply + accum-reduce on DVE (f32).
                    prod = dve_scr.tile([P, D], f32, tag="prod")
                    nc.vector.scalar_tensor_tensor(
                        out=prod[:],
                        in0=asub,
                        scalar=0.0,
                        in1=bsub,
                        op0=alu.add,
                        op1=alu.mult,
                        accum_out=p_t[:, t : t + 1],
                    )

                    # Sa always; Sb only for tail (non-Pool) chunks.
                    pairs = [(asub, sa_t)]
                    if not use_pool:
                        pairs.append((bsub, sb_t))
                    for sub, dst in pairs:
                        if _sq_on_dve(sq_idx, total_split_sq):
                            scr = dve_scr.tile([P, D], f32, tag="dsq")
                            nc.vector.scalar_tensor_tensor(
                                out=scr[:],
                                in0=sub,
                                scalar=0.0,
                                in1=sub,
                                op0=alu.add,
                                op1=alu.mult,
                                accum_out=dst[:, t : t + 1],
                            )
                        else:
                            pool_ = act_psum if ACT_SQ_PSUM else act_scr
                            scr = pool_.tile([P, D], f32, tag="asq")
                            nc.scalar.activation(
                                scr[:], sub, act.Square, accum_out=dst[:, t : t + 1]
                            )
                        sq_idx += 1

                tbase += chunk_t

                # Finalize ready column ranges early so only the last
                # chunk's finalize sits in the tail:
                #   out = P * sqrt(1 / (Sa * Sb)).
                if tbase in FIN_BOUNDS:
                    lo, hi = fin_lo, tbase
                    sb_src = sb_t
                    if sb_dt != f32:
                        nc.scalar.copy(sbf[:, lo:hi], sb_t[:, lo:hi])
                        sb_src = sbf
                    nc.vector.tensor_mul(
                        denom[:, lo:hi], sa_t[:, lo:hi], sb_src[:, lo:hi]
                    )
                    nc.vector.reciprocal(rec[:, lo:hi], denom[:, lo:hi])
                    nc.scalar.activation(rsq[:, lo:hi], rec[:, lo:hi], act.Sqrt)
                    nc.vector.tensor_mul(
                        out_t[:, lo:hi], p_t[:, lo:hi], rsq[:, lo:hi]
                    )
                    nc.sync.dma_start(o_d[:, lo:hi], out_t[:, lo:hi])
                    fin_lo = tbase

    return nc


def _get_nc():
    if "nc" not in _CACHE:
        _CACHE["nc"] = _build_bass()
    return _CACHE["nc"]


def kernel(a: np.ndarray, b: np.ndarray) -> np.ndarray:
    a = np.ascontiguousarray(np.asarray(a, dtype=np.float32)).reshape(ROWS, D)
    b = np.ascontiguousarray(np.asarray(b, dtype=np.float32)).reshape(ROWS, D)

    in_maps = []
    for c in range(N_CORES):
        sl = slice(c * ROWS_PER_CORE, (c + 1) * ROWS_PER_CORE)
        in_maps.append(
            {"a": a[sl].reshape(P, COLS), "b": b[sl].reshape(P, COLS)}
        )

    nc = _get_nc()
    res = bass_utils.run_bass_kernel_spmd(nc, in_maps, core_ids=list(range(N_CORES)))
    out = np.concatenate(
        [res.results[c]["out"].reshape(ROWS_PER_CORE) for c in range(N_CORES)]
    )
    return out.reshape(B, N)


# revision 10
# speedup vs baseline: 1.3034x; 1.0089x over previous
"""Rowwise cosine-similarity kernel for Trainium2 (8 NeuronCores, SPMD).

Computes out[b, n] = sum_d(an * bn) where an, bn are L2-normalized rows of
a, b [16, 4096, 256] -> out [16, 4096].

Sharding: 65536 rows split across 8 cores (8192 rows/core). Per core the
row slice is viewed as [128 partitions, 64 subtiles, 256], i.e. row
p*64 + t lives at partition p, subtile t. Everything is contiguous DMA.

Per 256-wide subtile the kernel needs three reductions over d:
  P  = sum(a*b),  Sa = sum(a*a),  Sb = sum(b*b)
then out = P * sqrt(1/(Sa*Sb)).

Engine split (the DMA stream, 16.8 MB/core at ~420 GB/s ~= 40.5 us, is the
roofline; every engine must stay under it):
  DVE : P via fused scalar_tensor_tensor (f32, 1 elem/cyc), a few Sa
        squares, plus segmented bf16 tensor_reduce of the Pool squares
        (2 elem/cyc in the 2x_1P perf mode - bf16 in AND out required).
  ACT : most Sa squares via Square+accumulate; finalize Sqrt + casts.
  Pool: (GpSimd) chunk-wide tensor_tensor b*b squares with bf16 output.
        Pool cannot reduce f32, so DVE picks up the cheap 2x reduce.
Sb only needs ~1e-3 relative accuracy (it feeds the normalization, so its
relative error passes straight through); bf16 is plenty. P must stay f32.

The last chunks are tapered and skip the Pool hop so the tail after the
final DMA bytes is short.
"""

import sys

if "/opt/trn_rl_repo" not in sys.path:
    sys.path.insert(0, "/opt/trn_rl_repo")

import numpy as np
import orjson

import concourse.bass as bass
import concourse.mybir as mybir
import concourse.tile as tile
from concourse import bass2jax, bass_utils

# ---------------------------------------------------------------------------
# Environment patches.
#
# 1. No cloud share in this sandbox: upload_artifacts would fail.
# 2. The walrus build here accepts at most ONE semaphore wait per
#    instruction; the Tile scheduler freely attaches several.  Post-process
#    the BIR before compiling: move surplus waits onto single-wait Drain
#    carrier instructions inserted just before the original instruction on
#    the same engine queue.
# ---------------------------------------------------------------------------

bass_utils.upload_artifacts = lambda tmpdir: ""

_MAX_WAITS = 1
REMOVE_PE = True


def _split_bir_waits(bir_json: bytes) -> bytes:
    d = orjson.loads(bir_json)
    ctr = 0
    for fn in d.get("functions", []):
        for blk in fn.get("blocks", []):
            insts = blk.get("instructions")
            if not insts:
                continue
            out = []
            for inst in insts:
                si = inst.get("sync_info")
                waits = (si or {}).get("on_wait") or []
                if len(waits) > _MAX_WAITS:
                    surplus = waits[:-_MAX_WAITS]
                    si["on_wait"] = waits[-_MAX_WAITS:]
                    for i in range(0, len(surplus), _MAX_WAITS):
                        out.append(
                            {
                                "name": f"WSPL-{ctr}",
                                "opcode": "Drain",
                                "engine": inst["engine"],
                                "ins": [],
                                "outs": [],
                                "is_reset_sema": False,
                                "debug": inst.get("debug", 0),
                                "sync_info": {
                                    "on_wait": surplus[i : i + _MAX_WAITS],
                                    "on_update": [],
                                },
                            }
                        )
                        ctr += 1
                out.append(inst)
            blk["instructions"] = out
    return orjson.dumps(d)


def _exempt_sp_from_entry_barrier(d: dict) -> None:
    """Let the SP (DMA-issuing) engine skip the kernel-entry barrier.

    The entry barrier only protects the const-AP memsets, which SP never
    reads; removing SP's blocking wait lets input DMAs start ~5 us earlier.
    The leader's release count is reduced so both sems still return to 0.
    """
    blk = d["functions"][0]["blocks"][0]
    insts = blk["instructions"]
    if not any(i.get("opcode") == "Memset" for i in insts):
        return
    sp_idx = None
    pool_add = None
    for i, inst in enumerate(insts):
        if inst.get("opcode") != "EventSemaphore":
            continue
        si = inst.get("sync_info") or {}
        ow = si.get("on_wait") or []
        ou = si.get("on_update") or []
        if not ou:
            continue
        u0 = ou[0]
        if "release" not in str(u0.get("ant_name", "")):
            continue
        if inst.get("engine") == "SP" and u0.get("update_mode") == "sem-dec":
            sp_idx = i
        if (
            inst.get("engine") == "Pool"
            and not ow
            and u0.get("update_mode") == "sem-add-imm"
        ):
            pool_add = inst
    if sp_idx is not None and pool_add is not None:
        uv = pool_add["sync_info"]["on_update"][0]
        if uv["update_value"] >= 2:
            del insts[sp_idx]
            uv["update_value"] -= 1


def _remove_pe_instructions(d: dict) -> None:
    """Drop every PE instruction from the BIR.

    This kernel never uses the tensor engine, but bass still emits barrier
    participation for it; the NEFF prolog then waits ~2.5 us for PE's
    HW-decode instruction stream to DMA in before the first barrier
    releases.  Removing PE from the program (and fixing the two barrier
    sems' counts) sidesteps that.
    """
    for fn in d.get("functions", []):
        for blk in fn.get("blocks", []):
            insts = blk.get("instructions") or []
            if not any(i.get("engine") == "PE" for i in insts):
                continue
            pe_gather = 0
            pe_release_waiters = 0
            for i in insts:
                if i.get("engine") != "PE":
                    continue
                si = i.get("sync_info") or {}
                for u in si.get("on_update") or []:
                    if "gather" in str(u.get("ant_name", "")):
                        pe_gather += 1
                if i.get("opcode") == "EventSemaphore":
                    for w in si.get("on_wait") or []:
                        if "release" in str(w.get("ant_name", "")):
                            pe_release_waiters += 1
            new = [i for i in insts if i.get("engine") != "PE"]
            for i in new:
                si = i.get("sync_info") or {}
                for w in si.get("on_wait") or []:
                    if (
                        "gather" in str(w.get("ant_name", ""))
                        and w.get("wait_mode") == "sem-ge-imm"
                    ):
                        w["wait_value"] -= pe_gather
                for u in si.get("on_update") or []:
                    if (
                        "gather" in str(u.get("ant_name", ""))
                        and u.get("update_mode") == "sem-sub-imm"
                    ):
                        u["update_value"] -= pe_gather
                    if (
                        "release" in str(u.get("ant_name", ""))
                        and u.get("update_mode") == "sem-add-imm"
                    ):
                        u["update_value"] -= pe_release_waiters
            blk["instructions"] = new


_orig_compile_bir_kernel = bass_utils.compile_bir_kernel


def _patched_compile_bir_kernel(bir_json, tmpdir, neff_name="file.neff"):
    if isinstance(bir_json, str):
        bir_json = bir_json.encode()
    d = orjson.loads(_split_bir_waits(bir_json))
    _exempt_sp_from_entry_barrier(d)
    if REMOVE_PE:
        _remove_pe_instructions(d)
    bir_json = orjson.dumps(d)
    return _orig_compile_bir_kernel(bir_json, tmpdir, neff_name=neff_name)


bass_utils.compile_bir_kernel = _patched_compile_bir_kernel
bass2jax.compile_bir_kernel = _patched_compile_bir_kernel

from concourse.vector_clock import ScopedClock  # noqa: E402


def _lean_drain_and_barrier(self, tick_clock, wait_clock):
    """Tile kernel tail without the trailing all-engine barrier.

    After the first barrier every engine is done with real work; gpsimd
    clears the semaphores and each engine halts independently (NRT waits
    for all engines anyway), so the second barrier only adds latency.
    """
    drain_inst = self.nc.sync.drain()
    wait_clock.add_sem_waits(
        drain_inst.ins, ScopedClock({None: tick_clock.global_clock})
    )
    self.nc.all_engine_barrier()
    popped = self.nc._tile_sem_poison_stack.pop()
    assert popped is self._sem_poison
    self.nc.clear_and_free_semaphores(list(self.sems.allocated().values()))


tile.TileContext._drain_and_barrier = _lean_drain_and_barrier

# ---------------------------------------------------------------------------
# Problem constants (hardcoded; kernel.py must be self-contained).
# ---------------------------------------------------------------------------

N_CORES = 8
B, N, D = 16, 4096, 256
ROWS = B * N                     # 65536
ROWS_PER_CORE = ROWS // N_CORES  # 8192
P = 128                          # SBUF partitions
T = ROWS_PER_CORE // P           # 64 subtiles per core
COLS = T * D                     # 16384 dram cols per partition
# Chunk sizes in subtiles. Uniform 8s while streaming; tapered at the end
# so the work that depends on the final bytes is tiny.
CHUNK_PLAN = (8, 8, 8, 8, 8, 8, 8, 4, 2, 2)
assert sum(CHUNK_PLAN) == T
# Chunks whose Sb squares run on Pool (GpSimd).  DISABLED (0): Pool's TT
# holds the shared SBUF port pair that DVE's 2-source stt also needs, so
# they serialize instead of overlapping (measured: stt slices stretch to
# 3.7 us while a Pool chunk-TT holds the port).
POOL_CHUNKS = 0
# Finalize boundaries (tbase values): emit out[lo:tb] when tbase hits these.
FIN_BOUNDS = (56, 62, 64)
CHUNK_BUFS = 8
BSQ_BUFS = 3
# First FILL_T subtiles of chunk 0 get their own small DMAs so compute
# starts as soon as they land instead of waiting for the full 2 MB pair.
FILL_T = 2

# The 128 square ops are split DVE/ACT.  Measured per-op: DVE stt ~344 ns,
# ACT square+accum ~596 ns (~554 with PSUM scratch).  Balance point:
# 22016 + 344x = 554(128-x) -> x ~= 54.
N_DVE_SQ = 58
ACT_SQ_PSUM = False


def _sq_on_dve(idx: int, total: int) -> bool:
    return (idx * N_DVE_SQ) // total != ((idx + 1) * N_DVE_SQ) // total


_CACHE: dict = {}


def _build_bass():
    f32 = mybir.dt.float32
    bf16 = mybir.dt.bfloat16
    alu = mybir.AluOpType
    act = mybir.ActivationFunctionType

    nc = bass.Bass(
        "TRN2",
        debug=False,
        num_devices=N_CORES,
        enable_asserts=False,
        enable_partition_id=False,
    )
    a_d = nc.dram_tensor("a", (P, COLS), f32, kind="ExternalInput").ap()
    b_d = nc.dram_tensor("b", (P, COLS), f32, kind="ExternalInput").ap()
    o_d = nc.dram_tensor("out", (P, T), f32, kind="ExternalOutput").ap()

    # Count non-Pool squares for the DVE/ACT interleave.
    n_pool_sq = sum(
        ct for i, ct in enumerate(CHUNK_PLAN) if i < POOL_CHUNKS
    )
    total_split_sq = 2 * T - n_pool_sq

    with tile.TileContext(nc) as tc, nc.allow_low_precision(
        reason="Sb feeds the normalization; bf16 (~1e-3 rel) is ample"
    ):
        with (
            tc.tile_pool(name="stats", bufs=1) as stats_pool,
            tc.tile_pool(name="chunks", bufs=CHUNK_BUFS) as chunk_pool,
            tc.tile_pool(name="bsq", bufs=BSQ_BUFS) as bsq_pool,
            tc.tile_pool(name="dscr", bufs=8) as dve_scr,
            tc.tile_pool(name="ascr", bufs=8) as act_scr,
            tc.tile_pool(name="apsc", bufs=2, space="PSUM") as act_psum,
            tc.tile_pool(name="fin", bufs=1) as fin_pool,
        ):
            p_t = stats_pool.tile([P, T], f32, tag="p")
            sa_t = stats_pool.tile([P, T], f32, tag="sa")
            sb_dt = bf16 if POOL_CHUNKS else f32
            sb_t = stats_pool.tile([P, T], sb_dt, tag="sb")
            sbf = fin_pool.tile([P, T], f32, tag="sbf")
            denom = fin_pool.tile([P, T], f32, tag="denom")
            rec = fin_pool.tile([P, T], f32, tag="rec")
            rsq = fin_pool.tile([P, T], f32, tag="rsq")
            out_t = fin_pool.tile([P, T], f32, tag="out")

            sq_idx = 0  # running index over DVE/ACT-split squares
            fin_lo = 0
            tbase = 0
            for ci, chunk_t in enumerate(CHUNK_PLAN):
                c0 = tbase * D
                c1 = (tbase + chunk_t) * D
                a_ch = chunk_pool.tile([P, CHUNK_PLAN[0] * D], f32, tag="a")
                b_ch = chunk_pool.tile([P, CHUNK_PLAN[0] * D], f32, tag="b")
                if ci == 0 and FILL_T:
                    fs = FILL_T * D
                    nc.sync.dma_start(a_ch[:, :fs], a_d[:, c0 : c0 + fs])
                    nc.sync.dma_start(b_ch[:, :fs], b_d[:, c0 : c0 + fs])
                    nc.sync.dma_start(a_ch[:, fs : chunk_t * D], a_d[:, c0 + fs : c1])
                    nc.sync.dma_start(b_ch[:, fs : chunk_t * D], b_d[:, c0 + fs : c1])
                else:
                    nc.sync.dma_start(a_ch[:, : chunk_t * D], a_d[:, c0:c1])
                    nc.sync.dma_start(b_ch[:, : chunk_t * D], b_d[:, c0:c1])

                use_pool = ci < POOL_CHUNKS
                if use_pool:
                    # Sb for the whole chunk: Pool squares (bf16 out), DVE
                    # segmented-reduces at 2 elem/cyc (bf16 in+out).
                    bsq = bsq_pool.tile([P, CHUNK_PLAN[0] * D], bf16, tag="bsq")
                    nc.gpsimd.tensor_tensor(
                        out=bsq[:, : chunk_t * D],
                        in0=b_ch[:, : chunk_t * D],
                        in1=b_ch[:, : chunk_t * D],
                        op=alu.mult,
                    )
                    nc.vector.tensor_reduce(
                        out=sb_t[:, tbase : tbase + chunk_t],
                        in_=bsq[:, : chunk_t * D].rearrange(
                            "p (t d) -> p t d", d=D
                        ),
                        axis=mybir.AxisListType.X,
                        op=alu.add,
                    )

                for s in range(chunk_t):
                    t = tbase + s
                    asub = a_ch[:, s * D : (s + 1) * D]
                    bsub = b_ch[:, s * D : (s + 1) * D]

                    # P: fused multiply + accum-reduce on DVE (f32).
                    prod = dve_scr.tile([P, D], f32, tag="prod")
                    nc.vector.scalar_tensor_tensor(
                        out=prod[:],
                        in0=asub,
                        scalar=0.0,
                        in1=bsub,
                        op0=alu.add,
                        op1=alu.mult,
                        accum_out=p_t[:, t : t + 1],
                    )

                    # Sa always; Sb only for tail (non-Pool) chunks.
                    pairs = [(asub, sa_t)]
                    if not use_pool:
                        pairs.append((bsub, sb_t))
                    for sub, dst in pairs:
                        if _sq_on_dve(sq_idx, total_split_sq):
                            scr = dve_scr.tile([P, D], f32, tag="dsq")
                            nc.vector.scalar_tensor_tensor(
                                out=scr[:],
                                in0=sub,
                                scalar=0.0,
                                in1=sub,
                                op0=alu.add,
                                op1=alu.mult,
                                accum_out=dst[:, t : t + 1],
                            )
                        else:
                            pool_ = act_psum if ACT_SQ_PSUM else act_scr
                            scr = pool_.tile([P, D], f32, tag="asq")
                            nc.scalar.activation(
                                scr[:], sub, act.Square, accum_out=dst[:, t : t + 1]
                            )
                        sq_idx += 1

                tbase += chunk_t

                # Finalize ready column ranges early so only the last
                # chunk's finalize sits in the tail:
                #   out = P * sqrt(1 / (Sa * Sb)).
                if tbase in FIN_BOUNDS:
                    lo, hi = fin_lo, tbase
                    sb_src = sb_t
                    if sb_dt != f32:
                        nc.scalar.copy(sbf[:, lo:hi], sb_t[:, lo:hi])
                        sb_src = sbf
                    nc.vector.tensor_mul(
                        denom[:, lo:hi], sa_t[:, lo:hi], sb_src[:, lo:hi]
                    )
                    nc.vector.reciprocal(rec[:, lo:hi], denom[:, lo:hi])
                    nc.scalar.activation(rsq[:, lo:hi], rec[:, lo:hi], act.Sqrt)
                    nc.vector.tensor_mul(
                        out_t[:, lo:hi], p_t[:, lo:hi], rsq[:, lo:hi]
                    )
                    nc.sync.dma_start(o_d[:, lo:hi], out_t[:, lo:hi])
                    fin_lo = tbase

    return nc


def _get_nc():
    if "nc" not in _CACHE:
        _CACHE["nc"] = _build_bass()
    return _CACHE["nc"]


def kernel(a: np.ndarray, b: np.ndarray) -> np.ndarray:
    a = np.ascontiguousarray(np.asarray(a, dtype=np.float32)).reshape(ROWS, D)
    b = np.ascontiguousarray(np.asarray(b, dtype=np.float32)).reshape(ROWS, D)

    in_maps = []
    for c in range(N_CORES):
        sl = slice(c * ROWS_PER_CORE, (c + 1) * ROWS_PER_CORE)
        in_maps.append(
            {"a": a[sl].reshape(P, COLS), "b": b[sl].reshape(P, COLS)}
        )

    nc = _get_nc()
    res = bass_utils.run_bass_kernel_spmd(nc, in_maps, core_ids=list(range(N_CORES)))
    out = np.concatenate(
        [res.results[c]["out"].reshape(ROWS_PER_CORE) for c in range(N_CORES)]
    )
    return out.reshape(B, N)


# revision 11
# speedup vs baseline: 1.3179x; 1.0111x over previous
"""Rowwise cosine-similarity kernel for Trainium2 (8 NeuronCores, SPMD).

Computes out[b, n] = sum_d(an * bn) where an, bn are L2-normalized rows of
a, b [16, 4096, 256] -> out [16, 4096].

Sharding: 65536 rows split across 8 cores (8192 rows/core). Per core the
row slice is viewed as [128 partitions, 64 subtiles, 256], i.e. row
p*64 + t lives at partition p, subtile t. Everything is contiguous DMA.

Per 256-wide subtile the kernel needs three reductions over d:
  P  = sum(a*b),  Sa = sum(a*a),  Sb = sum(b*b)
then out = P * sqrt(1/(Sa*Sb)).

Engine split (the DMA stream, 16.8 MB/core at ~420 GB/s ~= 40.5 us, is the
roofline; every engine must stay under it):
  DVE : P via fused scalar_tensor_tensor (f32, 1 elem/cyc), a few Sa
        squares, plus segmented bf16 tensor_reduce of the Pool squares
        (2 elem/cyc in the 2x_1P perf mode - bf16 in AND out required).
  ACT : most Sa squares via Square+accumulate; finalize Sqrt + casts.
  Pool: (GpSimd) chunk-wide tensor_tensor b*b squares with bf16 output.
        Pool cannot reduce f32, so DVE picks up the cheap 2x reduce.
Sb only needs ~1e-3 relative accuracy (it feeds the normalization, so its
relative error passes straight through); bf16 is plenty. P must stay f32.

The last chunks are tapered and skip the Pool hop so the tail after the
final DMA bytes is short.
"""

import sys

if "/opt/trn_rl_repo" not in sys.path:
    sys.path.insert(0, "/opt/trn_rl_repo")

import numpy as np
import orjson

import concourse.bass as bass
import concourse.mybir as mybir
import concourse.tile as tile
from concourse import bass2jax, bass_utils

# ---------------------------------------------------------------------------
# Environment patches.
#
# 1. No cloud share in this sandbox: upload_artifacts would fail.
# 2. The walrus build here accepts at most ONE semaphore wait per
#    instruction; the Tile scheduler freely attaches several.  Post-process
#    the BIR before compiling: move surplus waits onto single-wait Drain
#    carrier instructions inserted just before the original instruction on
#    the same engine queue.
# ---------------------------------------------------------------------------

bass_utils.upload_artifacts = lambda tmpdir: ""

_MAX_WAITS = 1
REMOVE_PE = False


def _split_bir_waits(bir_json: bytes) -> bytes:
    d = orjson.loads(bir_json)
    ctr = 0
    for fn in d.get("functions", []):
        for blk in fn.get("blocks", []):
            insts = blk.get("instructions")
            if not insts:
                continue
            out = []
            for inst in insts:
                si = inst.get("sync_info")
                waits = (si or {}).get("on_wait") or []
                if len(waits) > _MAX_WAITS:
                    surplus = waits[:-_MAX_WAITS]
                    si["on_wait"] = waits[-_MAX_WAITS:]
                    for i in range(0, len(surplus), _MAX_WAITS):
                        out.append(
                            {
                                "name": f"WSPL-{ctr}",
                                "opcode": "Drain",
                                "engine": inst["engine"],
                                "ins": [],
                                "outs": [],
                                "is_reset_sema": False,
                                "debug": inst.get("debug", 0),
                                "sync_info": {
                                    "on_wait": surplus[i : i + _MAX_WAITS],
                                    "on_update": [],
                                },
                            }
                        )
                        ctr += 1
                out.append(inst)
            blk["instructions"] = out
    return orjson.dumps(d)


def _exempt_sp_from_entry_barrier(d: dict) -> None:
    """Let the SP (DMA-issuing) engine skip the kernel-entry barrier.

    The entry barrier only protects the const-AP memsets, which SP never
    reads; removing SP's blocking wait lets input DMAs start ~5 us earlier.
    The leader's release count is reduced so both sems still return to 0.
    """
    blk = d["functions"][0]["blocks"][0]
    insts = blk["instructions"]
    if not any(i.get("opcode") == "Memset" for i in insts):
        return
    sp_idx = None
    pool_add = None
    for i, inst in enumerate(insts):
        if inst.get("opcode") != "EventSemaphore":
            continue
        si = inst.get("sync_info") or {}
        ow = si.get("on_wait") or []
        ou = si.get("on_update") or []
        if not ou:
            continue
        u0 = ou[0]
        if "release" not in str(u0.get("ant_name", "")):
            continue
        if inst.get("engine") == "SP" and u0.get("update_mode") == "sem-dec":
            sp_idx = i
        if (
            inst.get("engine") == "Pool"
            and not ow
            and u0.get("update_mode") == "sem-add-imm"
        ):
            pool_add = inst
    if sp_idx is not None and pool_add is not None:
        uv = pool_add["sync_info"]["on_update"][0]
        if uv["update_value"] >= 2:
            del insts[sp_idx]
            uv["update_value"] -= 1


def _remove_pe_instructions(d: dict) -> None:
    """Drop every PE instruction from the BIR.

    This kernel never uses the tensor engine, but bass still emits barrier
    participation for it; the NEFF prolog then waits ~2.5 us for PE's
    HW-decode instruction stream to DMA in before the first barrier
    releases.  Removing PE from the program (and fixing the two barrier
    sems' counts) sidesteps that.
    """
    for fn in d.get("functions", []):
        for blk in fn.get("blocks", []):
            insts = blk.get("instructions") or []
            if not any(i.get("engine") == "PE" for i in insts):
                continue
            pe_gather = 0
            pe_release_waiters = 0
            for i in insts:
                if i.get("engine") != "PE":
                    continue
                si = i.get("sync_info") or {}
                for u in si.get("on_update") or []:
                    if "gather" in str(u.get("ant_name", "")):
                        pe_gather += 1
                if i.get("opcode") == "EventSemaphore":
                    for w in si.get("on_wait") or []:
                        if "release" in str(w.get("ant_name", "")):
                            pe_release_waiters += 1
            new = [i for i in insts if i.get("engine") != "PE"]
            for i in new:
                si = i.get("sync_info") or {}
                for w in si.get("on_wait") or []:
                    if (
                        "gather" in str(w.get("ant_name", ""))
                        and w.get("wait_mode") == "sem-ge-imm"
                    ):
                        w["wait_value"] -= pe_gather
                for u in si.get("on_update") or []:
                    if (
                        "gather" in str(u.get("ant_name", ""))
                        and u.get("update_mode") == "sem-sub-imm"
                    ):
                        u["update_value"] -= pe_gather
                    if (
                        "release" in str(u.get("ant_name", ""))
                        and u.get("update_mode") == "sem-add-imm"
                    ):
                        u["update_value"] -= pe_release_waiters
            blk["instructions"] = new


_orig_compile_bir_kernel = bass_utils.compile_bir_kernel


def _patched_compile_bir_kernel(bir_json, tmpdir, neff_name="file.neff"):
    if isinstance(bir_json, str):
        bir_json = bir_json.encode()
    d = orjson.loads(_split_bir_waits(bir_json))
    _exempt_sp_from_entry_barrier(d)
    if REMOVE_PE:
        _remove_pe_instructions(d)
    bir_json = orjson.dumps(d)
    return _orig_compile_bir_kernel(bir_json, tmpdir, neff_name=neff_name)


bass_utils.compile_bir_kernel = _patched_compile_bir_kernel
bass2jax.compile_bir_kernel = _patched_compile_bir_kernel

from concourse.vector_clock import ScopedClock  # noqa: E402


def _lean_drain_and_barrier(self, tick_clock, wait_clock):
    """Tile kernel tail without the trailing all-engine barrier.

    After the first barrier every engine is done with real work; gpsimd
    clears the semaphores and each engine halts independently (NRT waits
    for all engines anyway), so the second barrier only adds latency.
    """
    drain_inst = self.nc.sync.drain()
    wait_clock.add_sem_waits(
        drain_inst.ins, ScopedClock({None: tick_clock.global_clock})
    )
    self.nc.all_engine_barrier()
    popped = self.nc._tile_sem_poison_stack.pop()
    assert popped is self._sem_poison
    self.nc.clear_and_free_semaphores(list(self.sems.allocated().values()))


tile.TileContext._drain_and_barrier = _lean_drain_and_barrier

# ---------------------------------------------------------------------------
# Problem constants (hardcoded; kernel.py must be self-contained).
# ---------------------------------------------------------------------------

N_CORES = 8
B, N, D = 16, 4096, 256
ROWS = B * N                     # 65536
ROWS_PER_CORE = ROWS // N_CORES  # 8192
P = 128                          # SBUF partitions
T = ROWS_PER_CORE // P           # 64 subtiles per core
COLS = T * D                     # 16384 dram cols per partition
# Chunk sizes in subtiles. Uniform 8s while streaming; tapered at the end
# so the work that depends on the final bytes is tiny.
CHUNK_PLAN = (8, 8, 8, 8, 8, 8, 8, 4, 2, 2)
assert sum(CHUNK_PLAN) == T
# Chunks whose Sb squares run on Pool (GpSimd).  DISABLED (0): Pool's TT
# holds the shared SBUF port pair that DVE's 2-source stt also needs, so
# they serialize instead of overlapping (measured: stt slices stretch to
# 3.7 us while a Pool chunk-TT holds the port).
POOL_CHUNKS = 0
# Finalize boundaries (tbase values): emit out[lo:tb] when tbase hits these.
FIN_BOUNDS = (56, 64)
CHUNK_BUFS = 6
BSQ_BUFS = 3
# First FILL_T subtiles of chunk 0 get their own small DMAs so compute
# starts as soon as they land instead of waiting for the full 2 MB pair.
FILL_T = 2

# The 128 square ops are split DVE/ACT.  Measured per-op: DVE stt ~344 ns,
# ACT square+accum ~596 ns (~554 with PSUM scratch).  Balance point:
# 22016 + 344x = 554(128-x) -> x ~= 54.
N_DVE_SQ = 58
ACT_SQ_PSUM = False


def _sq_on_dve(idx: int, total: int) -> bool:
    return (idx * N_DVE_SQ) // total != ((idx + 1) * N_DVE_SQ) // total


_CACHE: dict = {}


def _build_bass():
    f32 = mybir.dt.float32
    bf16 = mybir.dt.bfloat16
    alu = mybir.AluOpType
    act = mybir.ActivationFunctionType

    nc = bass.Bass(
        "TRN2",
        debug=False,
        num_devices=N_CORES,
        enable_asserts=False,
        enable_partition_id=False,
    )
    a_d = nc.dram_tensor("a", (P, COLS), f32, kind="ExternalInput").ap()
    b_d = nc.dram_tensor("b", (P, COLS), f32, kind="ExternalInput").ap()
    o_d = nc.dram_tensor("out", (P, T), f32, kind="ExternalOutput").ap()

    # Count non-Pool squares for the DVE/ACT interleave.
    n_pool_sq = sum(
        ct for i, ct in enumerate(CHUNK_PLAN) if i < POOL_CHUNKS
    )
    total_split_sq = 2 * T - n_pool_sq

    with tile.TileContext(nc) as tc, nc.allow_low_precision(
        reason="Sb feeds the normalization; bf16 (~1e-3 rel) is ample"
    ):
        with (
            tc.tile_pool(name="stats", bufs=1) as stats_pool,
            tc.tile_pool(name="chunks", bufs=CHUNK_BUFS) as chunk_pool,
            tc.tile_pool(name="bsq", bufs=BSQ_BUFS) as bsq_pool,
            tc.tile_pool(name="dscr", bufs=8) as dve_scr,
            tc.tile_pool(name="ascr", bufs=8) as act_scr,
            tc.tile_pool(name="apsc", bufs=2, space="PSUM") as act_psum,
            tc.tile_pool(name="fin", bufs=1) as fin_pool,
        ):
            p_t = stats_pool.tile([P, T], f32, tag="p")
            sa_t = stats_pool.tile([P, T], f32, tag="sa")
            sb_dt = bf16 if POOL_CHUNKS else f32
            sb_t = stats_pool.tile([P, T], sb_dt, tag="sb")
            sbf = fin_pool.tile([P, T], f32, tag="sbf")
            denom = fin_pool.tile([P, T], f32, tag="denom")
            rec = fin_pool.tile([P, T], f32, tag="rec")
            rsq = fin_pool.tile([P, T], f32, tag="rsq")
            out_t = fin_pool.tile([P, T], f32, tag="out")

            sq_idx = 0  # running index over DVE/ACT-split squares
            fin_lo = 0
            tbase = 0
            for ci, chunk_t in enumerate(CHUNK_PLAN):
                c0 = tbase * D
                c1 = (tbase + chunk_t) * D
                a_ch = chunk_pool.tile([P, CHUNK_PLAN[0] * D], f32, tag="a")
                b_ch = chunk_pool.tile([P, CHUNK_PLAN[0] * D], f32, tag="b")
                if ci == 0 and FILL_T:
                    fs = FILL_T * D
                    nc.sync.dma_start(a_ch[:, :fs], a_d[:, c0 : c0 + fs])
                    nc.sync.dma_start(b_ch[:, :fs], b_d[:, c0 : c0 + fs])
                    nc.sync.dma_start(a_ch[:, fs : chunk_t * D], a_d[:, c0 + fs : c1])
                    nc.sync.dma_start(b_ch[:, fs : chunk_t * D], b_d[:, c0 + fs : c1])
                else:
                    nc.sync.dma_start(a_ch[:, : chunk_t * D], a_d[:, c0:c1])
                    nc.sync.dma_start(b_ch[:, : chunk_t * D], b_d[:, c0:c1])

                use_pool = ci < POOL_CHUNKS
                if use_pool:
                    # Sb for the whole chunk: Pool squares (bf16 out), DVE
                    # segmented-reduces at 2 elem/cyc (bf16 in+out).
                    bsq = bsq_pool.tile([P, CHUNK_PLAN[0] * D], bf16, tag="bsq")
                    nc.gpsimd.tensor_tensor(
                        out=bsq[:, : chunk_t * D],
                        in0=b_ch[:, : chunk_t * D],
                        in1=b_ch[:, : chunk_t * D],
                        op=alu.mult,
                    )
                    nc.vector.tensor_reduce(
                        out=sb_t[:, tbase : tbase + chunk_t],
                        in_=bsq[:, : chunk_t * D].rearrange(
                            "p (t d) -> p t d", d=D
                        ),
                        axis=mybir.AxisListType.X,
                        op=alu.add,
                    )

                for s in range(chunk_t):
                    t = tbase + s
                    asub = a_ch[:, s * D : (s + 1) * D]
                    bsub = b_ch[:, s * D : (s + 1) * D]

                    # P: fused multiply + accum-reduce on DVE (f32).
                    prod = dve_scr.tile([P, D], f32, tag="prod")
                    nc.vector.scalar_tensor_tensor(
                        out=prod[:],
                        in0=asub,
                        scalar=0.0,
                        in1=bsub,
                        op0=alu.add,
                        op1=alu.mult,
                        accum_out=p_t[:, t : t + 1],
                    )

                    # Sa always; Sb only for tail (non-Pool) chunks.
                    pairs = [(asub, sa_t)]
                    if not use_pool:
                        pairs.append((bsub, sb_t))
                    for sub, dst in pairs:
                        if _sq_on_dve(sq_idx, total_split_sq):
                            scr = dve_scr.tile([P, D], f32, tag="dsq")
                            nc.vector.scalar_tensor_tensor(
                                out=scr[:],
                                in0=sub,
                                scalar=0.0,
                                in1=sub,
                                op0=alu.add,
                                op1=alu.mult,
                                accum_out=dst[:, t : t + 1],
                            )
                        else:
                            pool_ = act_psum if ACT_SQ_PSUM else act_scr
                            scr = pool_.tile([P, D], f32, tag="asq")
                            nc.scalar.activation(
                                scr[:], sub, act.Square, accum_out=dst[:, t : t + 1]
                            )
                        sq_idx += 1

                tbase += chunk_t

                # Finalize ready column ranges early so only the last
                # chunk's finalize sits in the tail:
                #   out = P * sqrt(1 / (Sa * Sb)).
                if tbase in FIN_BOUNDS:
                    lo, hi = fin_lo, tbase
                    sb_src = sb_t
                    if sb_dt != f32:
                        nc.scalar.copy(sbf[:, lo:hi], sb_t[:, lo:hi])
                        sb_src = sbf
                    nc.vector.tensor_mul(
                        denom[:, lo:hi], sa_t[:, lo:hi], sb_src[:, lo:hi]
                    )
                    nc.vector.reciprocal(rec[:, lo:hi], denom[:, lo:hi])
                    nc.scalar.activation(rsq[:, lo:hi], rec[:, lo:hi], act.Sqrt)
                    nc.vector.tensor_mul(
                        out_t[:, lo:hi], p_t[:, lo:hi], rsq[:, lo:hi]
                    )
                    nc.sync.dma_start(o_d[:, lo:hi], out_t[:, lo:hi])
                    fin_lo = tbase

    return nc


def _get_nc():
    if "nc" not in _CACHE:
        _CACHE["nc"] = _build_bass()
    return _CACHE["nc"]


def kernel(a: np.ndarray, b: np.ndarray) -> np.ndarray:
    a = np.ascontiguousarray(np.asarray(a, dtype=np.float32)).reshape(ROWS, D)
    b = np.ascontiguousarray(np.asarray(b, dtype=np.float32)).reshape(ROWS, D)

    in_maps = []
    for c in range(N_CORES):
        sl = slice(c * ROWS_PER_CORE, (c + 1) * ROWS_PER_CORE)
        in_maps.append(
            {"a": a[sl].reshape(P, COLS), "b": b[sl].reshape(P, COLS)}
        )

    nc = _get_nc()
    res = bass_utils.run_bass_kernel_spmd(nc, in_maps, core_ids=list(range(N_CORES)))
    out = np.concatenate(
        [res.results[c]["out"].reshape(ROWS_PER_CORE) for c in range(N_CORES)]
    )
    return out.reshape(B, N)
